# revision 6
# baseline (speedup 1.0000x reference)
"""Trainium2 Bass kernel for nn_MultiHeadAttention_70712341561681.

Math (faithful to the reference):
  slopes[h] = 2^(-h/2)
  q,k,v   = per-head projections of x (no bias)
  logits[b,h,i,j] = k_i . q_j - slopes[h]*|i-j|   (rows i = key time)
  masked to j <= i, weights = softmax over j      -> OUTPUT 1 [B,H,T,T]
  att[b,h,t,:] = weights[b,h,t,t] * v[b,h,t,:]
  out = concat_heads(att) @ Wp.T + bp             -> OUTPUT 0 [B,T,C]

Sharding: 8 cores = 2 batches x 4 head-groups (4 heads each). Each core
computes its 4 [T,T] weight slices (lower triangle only; upper left zero
by the runtime's pre-zeroed output buffers) and a partial out projection
over its heads; host sums the 4 partials per batch (the "all-reduce").

Device kernel per core:
  - Q^T,K^T per head [64,T] via fp32r matmuls against x^T; 3 extra rows
    carry a bf16-split of slopes[h]*j so the logits matmul directly
    produces k.q + slope*j (softmax-invariant form; the -slope*i part is
    applied per-partition as the exp() bias on the scalar engine).
  - per 128-row tile: matmul chunks (only j <= i+127), -1e30 tril mask on
    the diagonal 128-block, exp with row bias + accumulated row sum,
    reciprocal, normalize, diagonal extract via identity multiply-reduce.
  - att = diag * v (in place), PE transpose, out-proj matmul.
"""

import os
import sys

import numpy as np

for _p in ("/opt/trn_rl_repo", "/root/.axon_site/_ro/trn_rl_repo"):
    if os.path.isdir(_p) and _p not in sys.path:
        sys.path.append(_p)

import ml_dtypes
import concourse.bacc as bacc
import concourse.mybir as mybir
from concourse.tile import TileContext
from concourse.bass_utils import run_bass_kernel_spmd

B, T, C, H = 2, 2048, 1024, 16
HD = C // H            # 64
NCORES = 8
HPC = 4                # heads per core
NT = T // 128          # 16 row-tiles
NCC = C // 128         # 8 contraction chunks
F32 = mybir.dt.float32
F32R = mybir.dt.float32r
EXP = mybir.ActivationFunctionType.Exp
MUL = mybir.AluOpType.mult
ADD = mybir.AluOpType.add
NEG = -1.0e30


def _r(ap):
    return ap.bitcast(F32R)


def build_nc():
    nc = bacc.Bacc(trn_type="TRN2")
    xt = nc.dram_tensor("xt", [C, T], F32, kind="ExternalInput")
    wq = nc.dram_tensor("wq", [C, HPC * HD], F32, kind="ExternalInput")
    wk = nc.dram_tensor("wk", [C, HPC * HD], F32, kind="ExternalInput")
    wv = nc.dram_tensor("wv", [C, HPC * HD], F32, kind="ExternalInput")
    wp = nc.dram_tensor("wp", [HPC * HD, C], F32, kind="ExternalInput")
    brows = nc.dram_tensor("brows", [3 * HPC, T], F32, kind="ExternalInput")
    bsi = nc.dram_tensor("bsi", [128, HPC * NT], F32, kind="ExternalInput")
    mtri = nc.dram_tensor("mtri", [128, 128], F32, kind="ExternalInput")
    ident = nc.dram_tensor("ident", [128, 128], F32, kind="ExternalInput")
    ones3 = nc.dram_tensor("ones3", [3, T], F32, kind="ExternalInput")
    wout = nc.dram_tensor("wout", [HPC, T, T], F32, kind="ExternalOutput")
    pout = nc.dram_tensor("pout", [T, C], F32, kind="ExternalOutput")

    with TileContext(nc) as tc:
        with (
            tc.tile_pool(name="persist", bufs=1) as persist,
            tc.tile_pool(name="qpool", bufs=2) as qpool,
            tc.tile_pool(name="kpool", bufs=2) as kpool,
            tc.tile_pool(name="wpool", bufs=3) as wpool,
            tc.tile_pool(name="opool", bufs=2) as opool,
            tc.tile_pool(name="small", bufs=4) as small,
            tc.tile_pool(name="ttrp", bufs=2) as ttrp,
        ):
            # ---- resident loads ----
            xt_sb = persist.tile([128, NCC, T], F32R)
            nc.gpsimd.dma_start(out=xt_sb, in_=xt[:, :].rearrange("(cc p) t -> p cc t", p=128))
            wq_sb = persist.tile([128, NCC, HPC * HD], F32R)
            nc.gpsimd.dma_start(out=wq_sb, in_=wq[:, :].rearrange("(cc p) m -> p cc m", p=128))
            wk_sb = persist.tile([128, NCC, HPC * HD], F32R)
            nc.gpsimd.dma_start(out=wk_sb, in_=wk[:, :].rearrange("(cc p) m -> p cc m", p=128))
            wv_sb = persist.tile([128, NCC, HPC * HD], F32R)
            nc.gpsimd.dma_start(out=wv_sb, in_=wv[:, :].rearrange("(cc p) m -> p cc m", p=128))
            wp_sb = persist.tile([128, 2, C], F32R)
            nc.gpsimd.dma_start(out=wp_sb, in_=wp[:, :].rearrange("(hc p) co -> p hc co", p=128))
            bsi_sb = persist.tile([128, HPC * NT], F32)
            nc.sync.dma_start(out=bsi_sb, in_=bsi[:, :])
            mtri_sb = persist.tile([128, 128], F32)
            nc.sync.dma_start(out=mtri_sb, in_=mtri[:, :])
            ident_sb = persist.tile([128, 128], F32)
            nc.sync.dma_start(out=ident_sb, in_=ident[:, :])
            ident_r = persist.tile([128, 128], F32R)
            nc.gpsimd.dma_start(out=ident_r, in_=ident[:, :])
            v_sb = persist.tile([128, NT, HPC * HD], F32R)
            diag_sb = persist.tile([128, HPC * NT], F32)

            with tc.tile_pool(name="psA", bufs=2, space="PSUM") as psA:
                # ---- V for all 4 heads, t-major [t, hd] ----
                for tb in range(NT):
                    pv = psA.tile([128, HPC * HD], F32, tag="big")
                    for cc in range(NCC):
                        nc.tensor.matmul(
                            pv[:, :],
                            lhsT=(xt_sb[:, cc, tb * 128:(tb + 1) * 128]),
                            rhs=(wv_sb[:, cc, :]),
                            start=(cc == 0),
                            stop=(cc == NCC - 1),
                        )
                    nc.scalar.copy(out=v_sb[:, tb, :], in_=pv[:, :])

                # ---- per head: Q^T,K^T then the softmax rows ----
                for li in range(HPC):
                    qtext = qpool.tile([67, T], F32R, tag="qt")
                    ktext = kpool.tile([67, T], F32R, tag="kt")
                    nc.gpsimd.dma_start(out=qtext[64:67, :], in_=brows[3 * li:3 * li + 3, :])
                    nc.gpsimd.dma_start(out=ktext[64:67, :], in_=ones3[:, :])
                    for wsrc, dst, eng in ((wq_sb, qtext, "s"), (wk_sb, ktext, "v")):
                        ps = psA.tile([64, T], F32, tag="big")
                        for jc in range(T // 512):
                            for cc in range(NCC):
                                nc.tensor.matmul(
                                    ps[0:64, jc * 512:(jc + 1) * 512],
                                    lhsT=(wsrc[:, cc, li * HD:(li + 1) * HD]),
                                    rhs=(xt_sb[:, cc, jc * 512:(jc + 1) * 512]),
                                    start=(cc == 0),
                                    stop=(cc == NCC - 1),
                                )
                        if eng == "s":
                            nc.scalar.copy(out=dst[0:64, :], in_=ps[0:64, :])
                        else:
                            nc.vector.tensor_copy(dst[0:64, :], ps[0:64, :])

                    for k in range(NT):
                        i0 = k * 128
                        ln = i0 + 128
                        nch = (ln + 511) // 512
                        pg = psA.tile([128, T], F32, tag="big")
                        for jc in range(nch):
                            n0 = jc * 512
                            n1 = min(ln, n0 + 512)
                            nc.tensor.matmul(
                                pg[:, n0:n1],
                                lhsT=(ktext[0:67, i0:i0 + 128]),
                                rhs=(qtext[0:67, n0:n1]),
                                start=True,
                                stop=True,
                            )
                        # causal mask on the diagonal 128-block
                        nc.vector.tensor_add(pg[:, i0:ln], pg[:, i0:ln], mtri_sb[:, :])
                        wsb = wpool.tile([128, T], F32, tag="w")
                        ssum = small.tile([128, 1], F32, tag="ssum")
                        col = li * NT + k
                        nc.scalar.activation(
                            out=wsb[:, 0:ln],
                            in_=pg[:, 0:ln],
                            func=EXP,
                            bias=bsi_sb[:, col:col + 1],
                            scale=1.0,
                            accum_out=ssum[:, :],
                        )
                        rec = small.tile([128, 1], F32, tag="rec")
                        nc.vector.reciprocal(rec[:, :], ssum[:, :])
                        du = small.tile([128, 1], F32, tag="du")
                        scr = ttrp.tile([128, 128], F32, tag="scr")
                        nc.vector.tensor_mul(scr[:, :], wsb[:, i0:ln], ident_sb[:, :])
                        nc.vector.tensor_reduce(
                            out=du[:, :], in_=scr[:, :],
                            axis=mybir.AxisListType.X, op=ADD,
                        )
                        nc.vector.tensor_mul(diag_sb[:, col:col + 1], du[:, :], rec[:, :])
                        nc.vector.tensor_scalar_mul(wsb[:, 0:ln], wsb[:, 0:ln], rec[:, :])
                        nc.sync.dma_start(out=wout[li, i0:i0 + 128, 0:ln], in_=wsb[:, 0:ln])

                    # att = diag * v for this head (in place on v_sb)
                    for tb in range(NT):
                        nc.vector.tensor_scalar_mul(
                            v_sb[:, tb, li * HD:(li + 1) * HD],
                            v_sb[:, tb, li * HD:(li + 1) * HD],
                            diag_sb[:, li * NT + tb:li * NT + tb + 1],
                        )

            # ---- out projection: transpose att, then attT.T @ Wp rows ----
            with tc.tile_pool(name="psC", bufs=2, space="PSUM") as psC:
                attT = [
                    wpool.tile([128, T], F32R, tag="w", name=f"attT{i}")
                    for i in range(2)
                ]
                for tb in range(NT):
                    for hc in range(2):
                        pt = psC.tile([128, 128], F32R, tag="t")
                        nc.tensor.transpose(
                            pt[:, :],
                            in_=v_sb[:, tb, hc * 128:(hc + 1) * 128],
                            identity=ident_r[:, :],
                        )
                        nc.vector.tensor_copy(attT[hc][:, tb * 128:(tb + 1) * 128], pt[:, :])
                for tb in range(NT):
                    po = psC.tile([128, C], F32, tag="o")
                    for nn_ in range(2):
                        for hc in range(2):
                            nc.tensor.matmul(
                                po[:, nn_ * 512:(nn_ + 1) * 512],
                                lhsT=(attT[hc][:, tb * 128:(tb + 1) * 128]),
                                rhs=(wp_sb[:, hc, nn_ * 512:(nn_ + 1) * 512]),
                                start=(hc == 0),
                                stop=(hc == 1),
                            )
                    osb = opool.tile([128, C], F32, tag="osb")
                    nc.scalar.copy(out=osb[:, :], in_=po[:, :])
                    nc.sync.dma_start(out=pout[tb * 128:(tb + 1) * 128, :], in_=osb[:, :])

    nc.compile()
    return nc


def _bf16_split3(r):
    """r (f32 vector) -> 3 rows exactly representable in bf16 summing ~r."""
    hi = r.astype(ml_dtypes.bfloat16).astype(np.float32)
    lo = (r - hi).astype(ml_dtypes.bfloat16).astype(np.float32)
    lolo = (r - hi - lo).astype(ml_dtypes.bfloat16).astype(np.float32)
    return np.stack([hi, lo, lolo])


def core_heads(c):
    return [4 * (c % 4) + i for i in range(HPC)]


def make_in_maps(x, Wq, Wk, Wv, Wp):
    j = np.arange(T, dtype=np.float64)
    p = np.arange(128, dtype=np.float64)
    mtri = np.where(
        np.arange(128)[None, :] <= np.arange(128)[:, None], 0.0, NEG
    ).astype(np.float32)
    ident = np.eye(128, dtype=np.float32)
    in_maps = []
    for c in range(NCORES):
        b = c // 4
        hs = core_heads(c)
        xt = np.ascontiguousarray(x[b].T)
        wq_c = np.ascontiguousarray(np.concatenate([Wq[h].T for h in hs], axis=1))
        wk_c = np.ascontiguousarray(np.concatenate([Wk[h].T for h in hs], axis=1))
        wv_c = np.ascontiguousarray(np.concatenate([Wv[h].T for h in hs], axis=1))
        wp_c = np.ascontiguousarray(
            np.concatenate([Wp[:, h * HD:(h + 1) * HD].T for h in hs], axis=0)
        )
        brows = np.zeros((3 * HPC, T), np.float32)
        bsi = np.zeros((128, HPC * NT), np.float32)
        for li, h in enumerate(hs):
            slope = float(2.0 ** (h * (-8.0 / H)))
            brows[3 * li:3 * li + 3] = _bf16_split3((slope * j).astype(np.float32))
            for k in range(NT):
                bsi[:, li * NT + k] = (-slope * (128.0 * k + p)).astype(np.float32)
        in_maps.append(
            {
                "xt": xt,
                "wq": wq_c,
                "wk": wk_c,
                "wv": wv_c,
                "wp": wp_c,
                "brows": brows,
                "bsi": bsi,
                "mtri": mtri,
                "ident": ident,
                "ones3": np.ones((3, T), np.float32),
            }
        )
    return in_maps


def assemble(results, bp):
    weights = np.zeros((B, H, T, T), np.float32)
    out = np.zeros((B, T, C), np.float32)
    for c in range(NCORES):
        b = c // 4
        for li, h in enumerate(core_heads(c)):
            weights[b, h] = results[c]["wout"][li]
        out[b] += results[c]["pout"]
    out += np.asarray(bp, np.float32)[None, None, :]
    return out, weights


_NC_CACHE = None


def _get_nc():
    global _NC_CACHE
    if _NC_CACHE is None:
        _NC_CACHE = build_nc()
    return _NC_CACHE


def kernel(x, Wq, Wk, Wv, Wp, bp):
    x = np.asarray(x, np.float32)
    Wq = np.asarray(Wq, np.float32)
    Wk = np.asarray(Wk, np.float32)
    Wv = np.asarray(Wv, np.float32)
    Wp = np.asarray(Wp, np.float32)
    nc = _get_nc()
    in_maps = make_in_maps(x, Wq, Wk, Wv, Wp)
    res = run_bass_kernel_spmd(nc, in_maps, core_ids=list(range(NCORES)))
    return assemble(res.results, bp)


# revision 15
# speedup vs baseline: 1.0377x; 1.0377x over previous
"""Trainium2 Bass kernel for nn_MultiHeadAttention_70712341561681.

Math (faithful to the reference):
  slopes[h] = 2^(-h/2)
  q,k,v   = per-head projections of x (no bias)
  logits[b,h,i,j] = k_i . q_j - slopes[h]*|i-j|   (rows i = key time)
  masked to j <= i, weights = softmax over j      -> OUTPUT 1 [B,H,T,T]
  att[b,h,t,:] = weights[b,h,t,t] * v[b,h,t,:]
  out = concat_heads(att) @ Wp.T + bp             -> OUTPUT 0 [B,T,C]

Sharding: 8 cores = 2 batches x 4 head-groups (4 heads each). Each core
computes its 4 [T,T] weight slices (lower triangle only; upper left zero
by the runtime's pre-zeroed output buffers) and a partial out projection
over its heads; host sums the 4 partials per batch (the "all-reduce").

Device kernel per core:
  - Q^T,K^T per head [64,T] via fp32r matmuls against x^T; 3 extra rows
    carry a bf16-split of slopes[h]*j so the logits matmul directly
    produces k.q + slope*j (softmax-invariant form; the -slope*i part is
    applied per-partition as the exp() bias on the scalar engine).
  - per 128-row tile: matmul chunks (only j <= i+127), -1e30 tril mask on
    the diagonal 128-block, exp with row bias + accumulated row sum,
    reciprocal, normalize, diagonal extract via identity multiply-reduce.
  - att = diag * v (in place), PE transpose, out-proj matmul.
"""

import os
import sys

import numpy as np

for _p in ("/opt/trn_rl_repo", "/root/.axon_site/_ro/trn_rl_repo"):
    if os.path.isdir(_p) and _p not in sys.path:
        sys.path.append(_p)

import ml_dtypes
import concourse.bacc as bacc
import concourse.mybir as mybir
from concourse.tile import TileContext
from concourse.bass_utils import run_bass_kernel_spmd

B, T, C, H = 2, 2048, 1024, 16
HD = C // H            # 64
NCORES = 8
HPC = 4                # heads per core
NT = T // 128          # 16 row-tiles
NCC = C // 128         # 8 contraction chunks
F32 = mybir.dt.float32
F32R = mybir.dt.float32r
EXP = mybir.ActivationFunctionType.Exp
MUL = mybir.AluOpType.mult
ADD = mybir.AluOpType.add
NEG = -1.0e30


def _r(ap):
    return ap.bitcast(F32R)


def build_nc():
    nc = bacc.Bacc(trn_type="TRN2")
    # pre-swizzled on host to the exact SBUF layout: [partition, chunk, free]
    xt = nc.dram_tensor("xt", [128, NCC * T], F32R, kind="ExternalInput")
    wqk = nc.dram_tensor("wqk", [128, NCC * HPC * 128], F32R, kind="ExternalInput")
    wv = nc.dram_tensor("wv", [128, NCC * HPC * HD], F32R, kind="ExternalInput")
    wp = nc.dram_tensor("wp", [128, 2 * C], F32R, kind="ExternalInput")
    brows = nc.dram_tensor("brows", [3 * HPC, T], F32R, kind="ExternalInput")
    bsi = nc.dram_tensor("bsi", [128, HPC * NT], F32, kind="ExternalInput")
    mtri = nc.dram_tensor("mtri", [128, 128], F32, kind="ExternalInput")
    ident = nc.dram_tensor("ident", [128, 128], F32, kind="ExternalInput")
    ones3 = nc.dram_tensor("ones3", [3, T], F32R, kind="ExternalInput")
    identr = nc.dram_tensor("identr", [128, 128], F32R, kind="ExternalInput")
    wout = nc.dram_tensor("wout", [HPC, T, T], F32, kind="ExternalOutput")
    pout = nc.dram_tensor("pout", [T, C], F32, kind="ExternalOutput")

    with TileContext(nc) as tc:
        with (
            tc.tile_pool(name="persist", bufs=1) as persist,
            tc.tile_pool(name="qpool", bufs=2) as qpool,
            tc.tile_pool(name="kpool", bufs=2) as kpool,
            tc.tile_pool(name="wpool", bufs=3) as wpool,
            tc.tile_pool(name="opool", bufs=2) as opool,
            tc.tile_pool(name="small", bufs=6) as small,
            tc.tile_pool(name="ttrp", bufs=2) as ttrp,
        ):
            # ---- resident loads (weights on the scalar HWDGE queue so the
            # sync queue is free for the big xt load) ----
            wqk_sb = persist.tile([128, NCC, HPC * 128], F32R)
            nc.scalar.dma_start(out=wqk_sb, in_=wqk[:, :].rearrange("p (cc m) -> p cc m", cc=NCC))
            wv_sb = persist.tile([128, NCC, HPC * HD], F32R)
            nc.scalar.dma_start(out=wv_sb, in_=wv[:, :].rearrange("p (cc m) -> p cc m", cc=NCC))
            wp_sb = persist.tile([128, 2, C], F32R)
            nc.scalar.dma_start(out=wp_sb, in_=wp[:, :].rearrange("p (hc co) -> p hc co", hc=2))
            bsi_sb = persist.tile([128, HPC * NT], F32)
            nc.scalar.dma_start(out=bsi_sb, in_=bsi[:, :])
            mtri_sb = persist.tile([128, 128], F32)
            nc.scalar.dma_start(out=mtri_sb, in_=mtri[:, :])
            ident_sb = persist.tile([128, 128], F32)
            nc.scalar.dma_start(out=ident_sb, in_=ident[:, :])
            ident_r = persist.tile([128, 128], F32R)
            nc.scalar.dma_start(out=ident_r, in_=identr[:, :])
            xt_sb = persist.tile([128, NCC, T], F32R)
            nc.sync.dma_start(out=xt_sb, in_=xt[:, :].rearrange("p (cc t) -> p cc t", cc=NCC))
            v_sb = persist.tile([128, NT, HPC * HD], F32R)
            diag_sb = persist.tile([128, HPC * NT], F32)

            with tc.tile_pool(name="psA", bufs=4, space="PSUM") as psA:
                # ---- V for all 4 heads, t-major [t, hd] ----
                for tb in range(NT):
                    pv = psA.tile([128, HPC * HD], F32, tag="u")
                    for cc in range(NCC):
                        nc.tensor.matmul(
                            pv[:, :],
                            lhsT=xt_sb[:, cc, tb * 128:(tb + 1) * 128],
                            rhs=wv_sb[:, cc, :],
                            start=(cc == 0),
                            stop=(cc == NCC - 1),
                        )
                    nc.scalar.copy(out=v_sb[:, tb, :], in_=pv[:, :])

                # ---- per head: paired K|Q projection, then softmax rows ----
                for li in range(HPC):
                    qtext = qpool.tile([67, T], F32R, tag="qt")
                    ktext = kpool.tile([67, T], F32R, tag="kt")
                    qscr = qpool.tile([128, T], F32R, tag="qscr")
                    nc.sync.dma_start(out=qtext[64:67, :], in_=brows[3 * li:3 * li + 3, :])
                    nc.sync.dma_start(out=ktext[64:67, :], in_=ones3[:, :])
                    # psum rows 0-63 = K^T, rows 64-127 = Q^T; two 1024 halves
                    for h2 in range(2):
                        ps = psA.tile([128, T // 2], F32, tag="u")
                        for cc in range(NCC):
                            for j2 in range(2):
                                c0 = h2 * 1024 + j2 * 512
                                nc.tensor.matmul(
                                    ps[:, j2 * 512:(j2 + 1) * 512],
                                    lhsT=wqk_sb[:, cc, li * 128:(li + 1) * 128],
                                    rhs=xt_sb[:, cc, c0:c0 + 512],
                                    start=(cc == 0),
                                    stop=(cc == NCC - 1),
                                )
                        lo, hi = h2 * 1024, (h2 + 1) * 1024
                        if li % 2 == 0:
                            nc.scalar.copy(out=ktext[0:64, lo:hi], in_=ps[0:64, :])
                            nc.vector.tensor_copy(qscr[64:128, lo:hi], ps[64:128, :])
                        else:
                            nc.vector.tensor_copy(ktext[0:64, lo:hi], ps[0:64, :])
                            nc.scalar.copy(out=qscr[64:128, lo:hi], in_=ps[64:128, :])
                    # partition shift 64->0 via SBUF->SBUF DMA
                    nc.sync.dma_start(out=qtext[0:64, :], in_=qscr[64:128, :])

                    for k in range(NT):
                        i0 = k * 128
                        ln = i0 + 128
                        wsb = wpool.tile([128, T], F32, tag="w")
                        col = li * NT + k
                        nhalf = (ln + 1023) // 1024
                        ssum = small.tile([128, 2], F32, tag="ssum")
                        for hf in range(nhalf):
                            c0 = hf * 1024
                            c1 = min(ln, c0 + 1024)
                            pg = psA.tile([128, T // 2], F32, tag="u")
                            for jc in range((c1 - c0 + 511) // 512):
                                n0 = c0 + jc * 512
                                n1 = min(c1, n0 + 512)
                                nc.tensor.matmul(
                                    pg[:, n0 - c0:n1 - c0],
                                    lhsT=ktext[0:67, i0:i0 + 128],
                                    rhs=qtext[0:67, n0:n1],
                                    start=True,
                                    stop=True,
                                )
                            if i0 >= c0 and i0 < c1:
                                # causal mask on the diagonal 128-block
                                nc.vector.tensor_add(
                                    pg[:, i0 - c0:i0 - c0 + 128],
                                    pg[:, i0 - c0:i0 - c0 + 128],
                                    mtri_sb[:, :],
                                )
                            nc.scalar.activation(
                                out=wsb[:, c0:c1],
                                in_=pg[:, 0:c1 - c0],
                                func=EXP,
                                bias=bsi_sb[:, col:col + 1],
                                scale=1.0,
                                accum_out=ssum[:, hf:hf + 1],
                            )
                        rec = small.tile([128, 1], F32, tag="rec")
                        if nhalf == 2:
                            stot = small.tile([128, 1], F32, tag="stot")
                            nc.vector.tensor_add(stot[:, :], ssum[:, 0:1], ssum[:, 1:2])
                            nc.vector.reciprocal(rec[:, :], stot[:, :])
                        else:
                            nc.vector.reciprocal(rec[:, :], ssum[:, 0:1])
                        du = small.tile([128, 1], F32, tag="du")
                        scr = ttrp.tile([128, 128], F32, tag="scr")
                        # diag numerator from the unnormalized diag block
                        nc.vector.tensor_mul(scr[:, :], wsb[:, i0:ln], ident_sb[:, :])
                        nc.vector.tensor_scalar_mul(wsb[:, 0:ln], wsb[:, 0:ln], rec[:, :])
                        nc.gpsimd.dma_start(out=wout[li, i0:i0 + 128, 0:ln], in_=wsb[:, 0:ln])
                        nc.vector.tensor_reduce(
                            out=du[:, :], in_=scr[:, :],
                            axis=mybir.AxisListType.X, op=ADD,
                        )
                        nc.vector.tensor_mul(diag_sb[:, col:col + 1], du[:, :], rec[:, :])

                    # att = diag * v for this head (in place on v_sb)
                    for tb in range(NT):
                        nc.vector.tensor_scalar_mul(
                            v_sb[:, tb, li * HD:(li + 1) * HD],
                            v_sb[:, tb, li * HD:(li + 1) * HD],
                            diag_sb[:, li * NT + tb:li * NT + tb + 1],
                        )

            # ---- out projection: transpose att, then attT.T @ Wp rows ----
            with tc.tile_pool(name="psC", bufs=2, space="PSUM") as psC:
                attT = [
                    wpool.tile([128, T], F32R, tag="w", name=f"attT{i}")
                    for i in range(2)
                ]
                for tb in range(NT):
                    for hc in range(2):
                        pt = psC.tile([128, 128], F32R, tag="t")
                        nc.tensor.transpose(
                            pt[:, :],
                            in_=v_sb[:, tb, hc * 128:(hc + 1) * 128],
                            identity=ident_r[:, :],
                        )
                        nc.vector.tensor_copy(attT[hc][:, tb * 128:(tb + 1) * 128], pt[:, :])
                for tb in range(NT):
                    po = psC.tile([128, C], F32, tag="o")
                    for nn_ in range(2):
                        for hc in range(2):
                            nc.tensor.matmul(
                                po[:, nn_ * 512:(nn_ + 1) * 512],
                                lhsT=attT[hc][:, tb * 128:(tb + 1) * 128],
                                rhs=wp_sb[:, hc, nn_ * 512:(nn_ + 1) * 512],
                                start=(hc == 0),
                                stop=(hc == 1),
                            )
                    osb = opool.tile([128, C], F32, tag="osb")
                    if tb % 2 == 0:
                        nc.scalar.copy(out=osb[:, :], in_=po[:, :])
                    else:
                        nc.vector.tensor_copy(osb[:, :], po[:, :])
                    nc.sync.dma_start(out=pout[tb * 128:(tb + 1) * 128, :], in_=osb[:, :])

    nc.compile()
    return nc


def _bf16_split3(r):
    """r (f32 vector) -> 3 rows exactly representable in bf16 summing ~r."""
    hi = r.astype(ml_dtypes.bfloat16).astype(np.float32)
    lo = (r - hi).astype(ml_dtypes.bfloat16).astype(np.float32)
    lolo = (r - hi - lo).astype(ml_dtypes.bfloat16).astype(np.float32)
    return np.stack([hi, lo, lolo])


def core_heads(c):
    return [4 * (c % 4) + i for i in range(HPC)]


def make_in_maps(x, Wq, Wk, Wv, Wp):
    j = np.arange(T, dtype=np.float64)
    p = np.arange(128, dtype=np.float64)
    mtri = np.where(
        np.arange(128)[None, :] <= np.arange(128)[:, None], 0.0, NEG
    ).astype(np.float32)
    ident = np.eye(128, dtype=np.float32)
    in_maps = []
    for c in range(NCORES):
        b = c // 4
        hs = core_heads(c)
        def swz(a, nch):
            # [nch*128, F] -> [128, nch*F] matching SBUF [p, chunk, free]
            f = a.shape[1]
            return np.ascontiguousarray(
                a.reshape(nch, 128, f).transpose(1, 0, 2).reshape(128, nch * f)
            )

        xt = swz(x[b].T, NCC)
        wqk_c = swz(
            np.concatenate(
                [np.concatenate([Wk[h].T, Wq[h].T], axis=1) for h in hs], axis=1
            ),
            NCC,
        )
        wv_c = swz(np.concatenate([Wv[h].T for h in hs], axis=1), NCC)
        wp_c = swz(
            np.concatenate([Wp[:, h * HD:(h + 1) * HD].T for h in hs], axis=0), 2
        )
        brows = np.zeros((3 * HPC, T), np.float32)
        bsi = np.zeros((128, HPC * NT), np.float32)
        for li, h in enumerate(hs):
            slope = float(2.0 ** (h * (-8.0 / H)))
            brows[3 * li:3 * li + 3] = _bf16_split3((slope * j).astype(np.float32))
            for k in range(NT):
                bsi[:, li * NT + k] = (-slope * (128.0 * k + p)).astype(np.float32)
        in_maps.append(
            {
                "xt": xt,
                "wqk": wqk_c,
                "wv": wv_c,
                "wp": wp_c,
                "brows": brows,
                "bsi": bsi,
                "mtri": mtri,
                "ident": ident,
                "identr": ident,
                "ones3": np.ones((3, T), np.float32),
            }
        )
    return in_maps


def assemble(results, bp):
    weights = np.zeros((B, H, T, T), np.float32)
    out = np.zeros((B, T, C), np.float32)
    for c in range(NCORES):
        b = c // 4
        for li, h in enumerate(core_heads(c)):
            weights[b, h] = results[c]["wout"][li]
        out[b] += results[c]["pout"]
    out += np.asarray(bp, np.float32)[None, None, :]
    return out, weights


_NC_CACHE = None


def _get_nc():
    global _NC_CACHE
    if _NC_CACHE is None:
        _NC_CACHE = build_nc()
    return _NC_CACHE


def kernel(x, Wq, Wk, Wv, Wp, bp):
    x = np.asarray(x, np.float32)
    Wq = np.asarray(Wq, np.float32)
    Wk = np.asarray(Wk, np.float32)
    Wv = np.asarray(Wv, np.float32)
    Wp = np.asarray(Wp, np.float32)
    nc = _get_nc()
    in_maps = make_in_maps(x, Wq, Wk, Wv, Wp)
    res = run_bass_kernel_spmd(nc, in_maps, core_ids=list(range(NCORES)))
    return assemble(res.results, bp)


# revision 16
# speedup vs baseline: 1.1555x; 1.1135x over previous
"""Trainium2 Bass kernel for nn_MultiHeadAttention_70712341561681.

Math (faithful to the reference):
  slopes[h] = 2^(-h/2)
  q,k,v   = per-head projections of x (no bias)
  logits[b,h,i,j] = k_i . q_j - slopes[h]*|i-j|   (rows i = key time)
  masked to j <= i, weights = softmax over j      -> OUTPUT 1 [B,H,T,T]
  att[b,h,t,:] = weights[b,h,t,t] * v[b,h,t,:]
  out = concat_heads(att) @ Wp.T + bp             -> OUTPUT 0 [B,T,C]

Sharding: 8 cores = 2 batches x 4 head-groups (4 heads each). Each core
computes its 4 [T,T] weight slices (lower triangle only; upper left zero
by the runtime's pre-zeroed output buffers) and a partial out projection
over its heads; host sums the 4 partials per batch (the "all-reduce").

Device kernel per core:
  - Q^T,K^T per head [64,T] via fp32r matmuls against x^T; 3 extra rows
    carry a bf16-split of slopes[h]*j so the logits matmul directly
    produces k.q + slope*j (softmax-invariant form; the -slope*i part is
    applied per-partition as the exp() bias on the scalar engine).
  - per 128-row tile: matmul chunks (only j <= i+127), -1e30 tril mask on
    the diagonal 128-block, exp with row bias + accumulated row sum,
    reciprocal, normalize, diagonal extract via identity multiply-reduce.
  - att = diag * v (in place), PE transpose, out-proj matmul.
"""

import os
import sys

import numpy as np

for _p in ("/opt/trn_rl_repo", "/root/.axon_site/_ro/trn_rl_repo"):
    if os.path.isdir(_p) and _p not in sys.path:
        sys.path.append(_p)

import ml_dtypes
import concourse.bacc as bacc
import concourse.mybir as mybir
from concourse.tile import TileContext
from concourse.bass_utils import run_bass_kernel_spmd

B, T, C, H = 2, 2048, 1024, 16
HD = C // H            # 64
NCORES = 8
HPC = 4                # heads per core
NT = T // 128          # 16 row-tiles
NCC = C // 128         # 8 contraction chunks
F32 = mybir.dt.float32
F32R = mybir.dt.float32r
EXP = mybir.ActivationFunctionType.Exp
MUL = mybir.AluOpType.mult
ADD = mybir.AluOpType.add
NEG = -1.0e30


def _r(ap):
    return ap.bitcast(F32R)


def build_nc():
    nc = bacc.Bacc(trn_type="TRN2")
    # pre-swizzled on host to the exact SBUF layout: [partition, chunk, free]
    xt = nc.dram_tensor("xt", [128, NCC * T], F32R, kind="ExternalInput")
    wqk = nc.dram_tensor("wqk", [128, NCC * HPC * 128], F32R, kind="ExternalInput")
    wv = nc.dram_tensor("wv", [128, NCC * HPC * HD], F32R, kind="ExternalInput")
    wp = nc.dram_tensor("wp", [128, 2 * C], F32R, kind="ExternalInput")
    brows = nc.dram_tensor("brows", [3 * HPC, T], F32R, kind="ExternalInput")
    bsi = nc.dram_tensor("bsi", [128, HPC * NT], F32, kind="ExternalInput")
    mtri = nc.dram_tensor("mtri", [128, 128], F32, kind="ExternalInput")
    ident = nc.dram_tensor("ident", [128, 128], F32, kind="ExternalInput")
    ones3 = nc.dram_tensor("ones3", [3, T], F32R, kind="ExternalInput")
    identr = nc.dram_tensor("identr", [128, 128], F32R, kind="ExternalInput")
    wout = nc.dram_tensor("wout", [HPC, T, T], F32, kind="ExternalOutput")
    pout = nc.dram_tensor("pout", [T, C], F32, kind="ExternalOutput")

    with TileContext(nc) as tc:
        with (
            tc.tile_pool(name="persist", bufs=1) as persist,
            tc.tile_pool(name="qpool", bufs=2) as qpool,
            tc.tile_pool(name="qscrp", bufs=1) as qscrp,
            tc.tile_pool(name="kpool", bufs=2) as kpool,
            tc.tile_pool(name="wpool", bufs=4) as wpool,
            tc.tile_pool(name="opool", bufs=2) as opool,
            tc.tile_pool(name="small", bufs=6) as small,
            tc.tile_pool(name="ttrp", bufs=2) as ttrp,
        ):
            # ---- resident loads (weights on the scalar HWDGE queue so the
            # sync queue is free for the big xt load) ----
            wqk_sb = persist.tile([128, NCC, HPC * 128], F32R)
            nc.scalar.dma_start(out=wqk_sb, in_=wqk[:, :].rearrange("p (cc m) -> p cc m", cc=NCC))
            wv_sb = persist.tile([128, NCC, HPC * HD], F32R)
            nc.scalar.dma_start(out=wv_sb, in_=wv[:, :].rearrange("p (cc m) -> p cc m", cc=NCC))
            wp_sb = persist.tile([128, 2, C], F32R)
            nc.scalar.dma_start(out=wp_sb, in_=wp[:, :].rearrange("p (hc co) -> p hc co", hc=2))
            bsi_sb = persist.tile([128, HPC * NT], F32)
            nc.scalar.dma_start(out=bsi_sb, in_=bsi[:, :])
            mtri_sb = persist.tile([128, 128], F32)
            nc.scalar.dma_start(out=mtri_sb, in_=mtri[:, :])
            ident_sb = persist.tile([128, 128], F32)
            nc.scalar.dma_start(out=ident_sb, in_=ident[:, :])
            ident_r = persist.tile([128, 128], F32R)
            nc.scalar.dma_start(out=ident_r, in_=identr[:, :])
            xt_sb = persist.tile([128, NCC, T], F32R)
            nc.sync.dma_start(out=xt_sb, in_=xt[:, :].rearrange("p (cc t) -> p cc t", cc=NCC))
            v_sb = persist.tile([128, NT, HPC * HD], F32R)
            diag_sb = persist.tile([128, HPC * NT], F32)

            with tc.tile_pool(name="psA", bufs=4, space="PSUM") as psA:
                # ---- V for all 4 heads, t-major [t, hd] ----
                for tb in range(NT):
                    pv = psA.tile([128, HPC * HD], F32, tag="u")
                    for cc in range(NCC):
                        nc.tensor.matmul(
                            pv[:, :],
                            lhsT=xt_sb[:, cc, tb * 128:(tb + 1) * 128],
                            rhs=wv_sb[:, cc, :],
                            start=(cc == 0),
                            stop=(cc == NCC - 1),
                        )
                    nc.scalar.copy(out=v_sb[:, tb, :], in_=pv[:, :])

                # ---- per head: paired K|Q projection, then softmax rows ----
                for li in range(HPC):
                    qtext = qpool.tile([67, T], F32R, tag="qt")
                    ktext = kpool.tile([67, T], F32R, tag="kt")
                    qscr = qscrp.tile([128, T], F32R, tag="qscr")
                    nc.sync.dma_start(out=qtext[64:67, :], in_=brows[3 * li:3 * li + 3, :])
                    nc.sync.dma_start(out=ktext[64:67, :], in_=ones3[:, :])
                    # psum rows 0-63 = K^T, rows 64-127 = Q^T; two 1024 halves
                    for h2 in range(2):
                        ps = psA.tile([128, T // 2], F32, tag="u")
                        for cc in range(NCC):
                            for j2 in range(2):
                                c0 = h2 * 1024 + j2 * 512
                                nc.tensor.matmul(
                                    ps[:, j2 * 512:(j2 + 1) * 512],
                                    lhsT=wqk_sb[:, cc, li * 128:(li + 1) * 128],
                                    rhs=xt_sb[:, cc, c0:c0 + 512],
                                    start=(cc == 0),
                                    stop=(cc == NCC - 1),
                                )
                        lo, hi = h2 * 1024, (h2 + 1) * 1024
                        if li % 2 == 0:
                            nc.scalar.copy(out=ktext[0:64, lo:hi], in_=ps[0:64, :])
                            nc.vector.tensor_copy(qscr[64:128, lo:hi], ps[64:128, :])
                        else:
                            nc.vector.tensor_copy(ktext[0:64, lo:hi], ps[0:64, :])
                            nc.scalar.copy(out=qscr[64:128, lo:hi], in_=ps[64:128, :])
                        # partition shift 64->0 via SBUF->SBUF DMA, per half
                        nc.sync.dma_start(
                            out=qtext[0:64, lo:hi], in_=qscr[64:128, lo:hi]
                        )

                    for k in range(NT):
                        i0 = k * 128
                        ln = i0 + 128
                        wsb = wpool.tile([128, T], F32, tag="w")
                        col = li * NT + k
                        nhalf = (ln + 1023) // 1024
                        ssum = small.tile([128, 2], F32, tag="ssum")
                        for hf in range(nhalf):
                            c0 = hf * 1024
                            c1 = min(ln, c0 + 1024)
                            pg = psA.tile([128, T // 2], F32, tag="u")
                            for jc in range((c1 - c0 + 511) // 512):
                                n0 = c0 + jc * 512
                                n1 = min(c1, n0 + 512)
                                nc.tensor.matmul(
                                    pg[:, n0 - c0:n1 - c0],
                                    lhsT=ktext[0:67, i0:i0 + 128],
                                    rhs=qtext[0:67, n0:n1],
                                    start=True,
                                    stop=True,
                                )
                            if i0 >= c0 and i0 < c1:
                                # causal mask on the diagonal 128-block
                                nc.vector.tensor_add(
                                    pg[:, i0 - c0:i0 - c0 + 128],
                                    pg[:, i0 - c0:i0 - c0 + 128],
                                    mtri_sb[:, :],
                                )
                            nc.scalar.activation(
                                out=wsb[:, c0:c1],
                                in_=pg[:, 0:c1 - c0],
                                func=EXP,
                                bias=bsi_sb[:, col:col + 1],
                                scale=1.0,
                                accum_out=ssum[:, hf:hf + 1],
                            )
                        rec = small.tile([128, 1], F32, tag="rec")
                        if nhalf == 2:
                            stot = small.tile([128, 1], F32, tag="stot")
                            nc.vector.tensor_add(stot[:, :], ssum[:, 0:1], ssum[:, 1:2])
                            nc.vector.reciprocal(rec[:, :], stot[:, :])
                        else:
                            nc.vector.reciprocal(rec[:, :], ssum[:, 0:1])
                        du = small.tile([128, 1], F32, tag="du")
                        scr = ttrp.tile([128, 128], F32, tag="scr")
                        # diag numerator from the unnormalized diag block
                        nc.vector.tensor_mul(scr[:, :], wsb[:, i0:ln], ident_sb[:, :])
                        nc.vector.tensor_scalar_mul(wsb[:, 0:ln], wsb[:, 0:ln], rec[:, :])
                        nc.gpsimd.dma_start(out=wout[li, i0:i0 + 128, 0:ln], in_=wsb[:, 0:ln])
                        nc.vector.tensor_reduce(
                            out=du[:, :], in_=scr[:, :],
                            axis=mybir.AxisListType.X, op=ADD,
                        )
                        nc.vector.tensor_mul(diag_sb[:, col:col + 1], du[:, :], rec[:, :])

                    # att = diag * v for this head (in place on v_sb)
                    for tb in range(NT):
                        nc.vector.tensor_scalar_mul(
                            v_sb[:, tb, li * HD:(li + 1) * HD],
                            v_sb[:, tb, li * HD:(li + 1) * HD],
                            diag_sb[:, li * NT + tb:li * NT + tb + 1],
                        )

            # ---- out projection: transpose att, then attT.T @ Wp rows ----
            with tc.tile_pool(name="psC", bufs=2, space="PSUM") as psC:
                attT = [
                    wpool.tile([128, T], F32R, tag="w", name=f"attT{i}")
                    for i in range(2)
                ]
                for tb in range(NT):
                    for hc in range(2):
                        pt = psC.tile([128, 128], F32R, tag="t")
                        nc.tensor.transpose(
                            pt[:, :],
                            in_=v_sb[:, tb, hc * 128:(hc + 1) * 128],
                            identity=ident_r[:, :],
                        )
                        nc.vector.tensor_copy(attT[hc][:, tb * 128:(tb + 1) * 128], pt[:, :])
                for tb in range(NT):
                    po = psC.tile([128, C], F32, tag="o")
                    for nn_ in range(2):
                        for hc in range(2):
                            nc.tensor.matmul(
                                po[:, nn_ * 512:(nn_ + 1) * 512],
                                lhsT=attT[hc][:, tb * 128:(tb + 1) * 128],
                                rhs=wp_sb[:, hc, nn_ * 512:(nn_ + 1) * 512],
                                start=(hc == 0),
                                stop=(hc == 1),
                            )
                    osb = opool.tile([128, C], F32, tag="osb")
                    if tb % 2 == 0:
                        nc.scalar.copy(out=osb[:, :], in_=po[:, :])
                    else:
                        nc.vector.tensor_copy(osb[:, :], po[:, :])
                    nc.sync.dma_start(out=pout[tb * 128:(tb + 1) * 128, :], in_=osb[:, :])

    nc.compile()
    return nc


def _bf16_split3(r):
    """r (f32 vector) -> 3 rows exactly representable in bf16 summing ~r."""
    hi = r.astype(ml_dtypes.bfloat16).astype(np.float32)
    lo = (r - hi).astype(ml_dtypes.bfloat16).astype(np.float32)
    lolo = (r - hi - lo).astype(ml_dtypes.bfloat16).astype(np.float32)
    return np.stack([hi, lo, lolo])


def core_heads(c):
    return [4 * (c % 4) + i for i in range(HPC)]


def make_in_maps(x, Wq, Wk, Wv, Wp):
    j = np.arange(T, dtype=np.float64)
    p = np.arange(128, dtype=np.float64)
    mtri = np.where(
        np.arange(128)[None, :] <= np.arange(128)[:, None], 0.0, NEG
    ).astype(np.float32)
    ident = np.eye(128, dtype=np.float32)
    in_maps = []
    for c in range(NCORES):
        b = c // 4
        hs = core_heads(c)
        def swz(a, nch):
            # [nch*128, F] -> [128, nch*F] matching SBUF [p, chunk, free]
            f = a.shape[1]
            return np.ascontiguousarray(
                a.reshape(nch, 128, f).transpose(1, 0, 2).reshape(128, nch * f)
            )

        xt = swz(x[b].T, NCC)
        wqk_c = swz(
            np.concatenate(
                [np.concatenate([Wk[h].T, Wq[h].T], axis=1) for h in hs], axis=1
            ),
            NCC,
        )
        wv_c = swz(np.concatenate([Wv[h].T for h in hs], axis=1), NCC)
        wp_c = swz(
            np.concatenate([Wp[:, h * HD:(h + 1) * HD].T for h in hs], axis=0), 2
        )
        brows = np.zeros((3 * HPC, T), np.float32)
        bsi = np.zeros((128, HPC * NT), np.float32)
        for li, h in enumerate(hs):
            slope = float(2.0 ** (h * (-8.0 / H)))
            brows[3 * li:3 * li + 3] = _bf16_split3((slope * j).astype(np.float32))
            for k in range(NT):
                bsi[:, li * NT + k] = (-slope * (128.0 * k + p)).astype(np.float32)
        in_maps.append(
            {
                "xt": xt,
                "wqk": wqk_c,
                "wv": wv_c,
                "wp": wp_c,
                "brows": brows,
                "bsi": bsi,
                "mtri": mtri,
                "ident": ident,
                "identr": ident,
                "ones3": np.ones((3, T), np.float32),
            }
        )
    return in_maps


def assemble(results, bp):
    weights = np.zeros((B, H, T, T), np.float32)
    out = np.zeros((B, T, C), np.float32)
    for c in range(NCORES):
        b = c // 4
        for li, h in enumerate(core_heads(c)):
            weights[b, h] = results[c]["wout"][li]
        out[b] += results[c]["pout"]
    out += np.asarray(bp, np.float32)[None, None, :]
    return out, weights


_NC_CACHE = None


def _get_nc():
    global _NC_CACHE
    if _NC_CACHE is None:
        _NC_CACHE = build_nc()
    return _NC_CACHE


def kernel(x, Wq, Wk, Wv, Wp, bp):
    x = np.asarray(x, np.float32)
    Wq = np.asarray(Wq, np.float32)
    Wk = np.asarray(Wk, np.float32)
    Wv = np.asarray(Wv, np.float32)
    Wp = np.asarray(Wp, np.float32)
    nc = _get_nc()
    in_maps = make_in_maps(x, Wq, Wk, Wv, Wp)
    res = run_bass_kernel_spmd(nc, in_maps, core_ids=list(range(NCORES)))
    return assemble(res.results, bp)


# revision 17
# speedup vs baseline: 1.1870x; 1.0272x over previous
"""Trainium2 Bass kernel for nn_MultiHeadAttention_70712341561681.

Math (faithful to the reference):
  slopes[h] = 2^(-h/2)
  q,k,v   = per-head projections of x (no bias)
  logits[b,h,i,j] = k_i . q_j - slopes[h]*|i-j|   (rows i = key time)
  masked to j <= i, weights = softmax over j      -> OUTPUT 1 [B,H,T,T]
  att[b,h,t,:] = weights[b,h,t,t] * v[b,h,t,:]
  out = concat_heads(att) @ Wp.T + bp             -> OUTPUT 0 [B,T,C]

Sharding: 8 cores = 2 batches x 4 head-groups (4 heads each). Each core
computes its 4 [T,T] weight slices (lower triangle only; upper left zero
by the runtime's pre-zeroed output buffers) and a partial out projection
over its heads; host sums the 4 partials per batch (the "all-reduce").

Device kernel per core:
  - Q^T,K^T per head [64,T] via fp32r matmuls against x^T; 3 extra rows
    carry a bf16-split of slopes[h]*j so the logits matmul directly
    produces k.q + slope*j (softmax-invariant form; the -slope*i part is
    applied per-partition as the exp() bias on the scalar engine).
  - per 128-row tile: matmul chunks (only j <= i+127), -1e30 tril mask on
    the diagonal 128-block, exp with row bias + accumulated row sum,
    reciprocal, normalize, diagonal extract via identity multiply-reduce.
  - att = diag * v (in place), PE transpose, out-proj matmul.
"""

import os
import sys

import numpy as np

for _p in ("/opt/trn_rl_repo", "/root/.axon_site/_ro/trn_rl_repo"):
    if os.path.isdir(_p) and _p not in sys.path:
        sys.path.append(_p)

import ml_dtypes
import concourse.bacc as bacc
import concourse.mybir as mybir
from concourse.tile import TileContext
from concourse.bass_utils import run_bass_kernel_spmd

B, T, C, H = 2, 2048, 1024, 16
HD = C // H            # 64
NCORES = 8
HPC = 4                # heads per core
NT = T // 128          # 16 row-tiles
NCC = C // 128         # 8 contraction chunks
F32 = mybir.dt.float32
F32R = mybir.dt.float32r
EXP = mybir.ActivationFunctionType.Exp
MUL = mybir.AluOpType.mult
ADD = mybir.AluOpType.add
NEG = -1.0e30


def _r(ap):
    return ap.bitcast(F32R)


def build_nc():
    nc = bacc.Bacc(trn_type="TRN2")
    # pre-swizzled on host to the exact SBUF layout: [partition, chunk, free]
    xt = nc.dram_tensor("xt", [128, NCC * T], F32R, kind="ExternalInput")
    wqk = nc.dram_tensor("wqk", [128, NCC * HPC * 128], F32R, kind="ExternalInput")
    wv = nc.dram_tensor("wv", [128, NCC * HPC * HD], F32R, kind="ExternalInput")
    wp = nc.dram_tensor("wp", [128, 2 * C], F32R, kind="ExternalInput")
    brows = nc.dram_tensor("brows", [3 * HPC, T], F32R, kind="ExternalInput")
    bsi = nc.dram_tensor("bsi", [128, HPC * NT], F32, kind="ExternalInput")
    mtri = nc.dram_tensor("mtri", [128, 128], F32, kind="ExternalInput")
    ident = nc.dram_tensor("ident", [128, 128], F32, kind="ExternalInput")
    ones3 = nc.dram_tensor("ones3", [3, T], F32R, kind="ExternalInput")
    identr = nc.dram_tensor("identr", [128, 128], F32R, kind="ExternalInput")
    wout = nc.dram_tensor("wout", [HPC, T, T], F32, kind="ExternalOutput")
    pout = nc.dram_tensor("pout", [T, C], F32, kind="ExternalOutput")

    with TileContext(nc) as tc:
        with (
            tc.tile_pool(name="persist", bufs=1) as persist,
            tc.tile_pool(name="qpool", bufs=2) as qpool,
            tc.tile_pool(name="qscrp", bufs=1) as qscrp,
            tc.tile_pool(name="kpool", bufs=2) as kpool,
            tc.tile_pool(name="wpool", bufs=4) as wpool,
            tc.tile_pool(name="opool", bufs=2) as opool,
            tc.tile_pool(name="small", bufs=6) as small,
            tc.tile_pool(name="ttrp", bufs=2) as ttrp,
        ):
            # ---- resident loads (weights on the scalar HWDGE queue so the
            # sync queue is free for the big xt load) ----
            wqk_sb = persist.tile([128, NCC, HPC * 128], F32R)
            nc.scalar.dma_start(out=wqk_sb, in_=wqk[:, :].rearrange("p (cc m) -> p cc m", cc=NCC))
            wv_sb = persist.tile([128, NCC, HPC * HD], F32R)
            nc.scalar.dma_start(out=wv_sb, in_=wv[:, :].rearrange("p (cc m) -> p cc m", cc=NCC))
            wp_sb = persist.tile([128, 2, C], F32R)
            nc.scalar.dma_start(out=wp_sb, in_=wp[:, :].rearrange("p (hc co) -> p hc co", hc=2))
            bsi_sb = persist.tile([128, HPC * NT], F32)
            nc.scalar.dma_start(out=bsi_sb, in_=bsi[:, :])
            mtri_sb = persist.tile([128, 128], F32)
            nc.scalar.dma_start(out=mtri_sb, in_=mtri[:, :])
            ident_sb = persist.tile([128, 128], F32)
            nc.scalar.dma_start(out=ident_sb, in_=ident[:, :])
            ident_r = persist.tile([128, 128], F32R)
            nc.scalar.dma_start(out=ident_r, in_=identr[:, :])
            xt_sb = persist.tile([128, NCC, T], F32R)
            nc.sync.dma_start(out=xt_sb, in_=xt[:, :].rearrange("p (cc t) -> p cc t", cc=NCC))
            v_sb = persist.tile([128, NT, HPC * HD], F32R)
            diag_sb = persist.tile([128, HPC * NT], F32)

            with tc.tile_pool(name="psA", bufs=4, space="PSUM") as psA:

                def emit_qk(li):
                    qtext = qpool.tile([67, T], F32R, tag="qt", name=f"qt{li}")
                    ktext = kpool.tile([67, T], F32R, tag="kt", name=f"kt{li}")
                    qscr = qscrp.tile([128, T], F32R, tag="qscr", name=f"qs{li}")
                    nc.sync.dma_start(out=qtext[64:67, :], in_=brows[3 * li:3 * li + 3, :])
                    nc.sync.dma_start(out=ktext[64:67, :], in_=ones3[:, :])
                    # psum rows 0-63 = K^T, rows 64-127 = Q^T; two 1024 halves
                    for h2 in range(2):
                        ps = psA.tile([128, T // 2], F32, tag="u", name=f"ps{li}_{h2}")
                        for cc in range(NCC):
                            for j2 in range(2):
                                c0 = h2 * 1024 + j2 * 512
                                nc.tensor.matmul(
                                    ps[:, j2 * 512:(j2 + 1) * 512],
                                    lhsT=wqk_sb[:, cc, li * 128:(li + 1) * 128],
                                    rhs=xt_sb[:, cc, c0:c0 + 512],
                                    start=(cc == 0),
                                    stop=(cc == NCC - 1),
                                )
                        lo, hi = h2 * 1024, (h2 + 1) * 1024
                        if li % 2 == 0:
                            nc.scalar.copy(out=ktext[0:64, lo:hi], in_=ps[0:64, :])
                            nc.vector.tensor_copy(qscr[64:128, lo:hi], ps[64:128, :])
                        else:
                            nc.vector.tensor_copy(ktext[0:64, lo:hi], ps[0:64, :])
                            nc.scalar.copy(out=qscr[64:128, lo:hi], in_=ps[64:128, :])
                        # partition shift 64->0 via SBUF->SBUF DMA, per half
                        nc.sync.dma_start(
                            out=qtext[0:64, lo:hi], in_=qscr[64:128, lo:hi]
                        )
                    return qtext, ktext

                def emit_b(li, qtext, ktext):
                    for k in range(NT):
                        i0 = k * 128
                        ln = i0 + 128
                        wsb = wpool.tile([128, T], F32, tag="w", name=f"w{li}_{k}")
                        col = li * NT + k
                        nhalf = (ln + 1023) // 1024
                        ssum = small.tile([128, 2], F32, tag="ssum", name=f"ss{li}_{k}")
                        for hf in range(nhalf):
                            c0 = hf * 1024
                            c1 = min(ln, c0 + 1024)
                            pg = psA.tile([128, T // 2], F32, tag="u", name=f"pg{li}_{k}_{hf}")
                            for jc in range((c1 - c0 + 511) // 512):
                                n0 = c0 + jc * 512
                                n1 = min(c1, n0 + 512)
                                nc.tensor.matmul(
                                    pg[:, n0 - c0:n1 - c0],
                                    lhsT=ktext[0:67, i0:i0 + 128],
                                    rhs=qtext[0:67, n0:n1],
                                    start=True,
                                    stop=True,
                                )
                            if i0 >= c0 and i0 < c1:
                                # causal mask on the diagonal 128-block
                                nc.vector.tensor_add(
                                    pg[:, i0 - c0:i0 - c0 + 128],
                                    pg[:, i0 - c0:i0 - c0 + 128],
                                    mtri_sb[:, :],
                                )
                            nc.scalar.activation(
                                out=wsb[:, c0:c1],
                                in_=pg[:, 0:c1 - c0],
                                func=EXP,
                                bias=bsi_sb[:, col:col + 1],
                                scale=1.0,
                                accum_out=ssum[:, hf:hf + 1],
                            )
                        rec = small.tile([128, 1], F32, tag="rec", name=f"rc{li}_{k}")
                        if nhalf == 2:
                            stot = small.tile([128, 1], F32, tag="stot", name=f"st{li}_{k}")
                            nc.vector.tensor_add(stot[:, :], ssum[:, 0:1], ssum[:, 1:2])
                            nc.vector.reciprocal(rec[:, :], stot[:, :])
                        else:
                            nc.vector.reciprocal(rec[:, :], ssum[:, 0:1])
                        du = small.tile([128, 1], F32, tag="du", name=f"du{li}_{k}")
                        scr = ttrp.tile([128, 128], F32, tag="scr", name=f"sc{li}_{k}")
                        # diag numerator from the unnormalized diag block
                        nc.vector.tensor_mul(scr[:, :], wsb[:, i0:ln], ident_sb[:, :])
                        nc.vector.tensor_scalar_mul(wsb[:, 0:ln], wsb[:, 0:ln], rec[:, :])
                        nc.gpsimd.dma_start(out=wout[li, i0:i0 + 128, 0:ln], in_=wsb[:, 0:ln])
                        nc.vector.tensor_reduce(
                            out=du[:, :], in_=scr[:, :],
                            axis=mybir.AxisListType.X, op=ADD,
                        )
                        nc.vector.tensor_mul(diag_sb[:, col:col + 1], du[:, :], rec[:, :])

                    # att = diag * v for this head (in place on v_sb)
                    for tb in range(NT):
                        nc.vector.tensor_scalar_mul(
                            v_sb[:, tb, li * HD:(li + 1) * HD],
                            v_sb[:, tb, li * HD:(li + 1) * HD],
                            diag_sb[:, li * NT + tb:li * NT + tb + 1],
                        )

                # head 0's Q/K first so its softmax chain starts immediately;
                # the V projection then fills PE gaps during head 0's
                # vector/scalar-heavy phase.
                qk0 = emit_qk(0)
                for tb in range(NT):
                    pv = psA.tile([128, HPC * HD], F32, tag="u", name=f"pv{tb}")
                    for cc in range(NCC):
                        nc.tensor.matmul(
                            pv[:, :],
                            lhsT=xt_sb[:, cc, tb * 128:(tb + 1) * 128],
                            rhs=wv_sb[:, cc, :],
                            start=(cc == 0),
                            stop=(cc == NCC - 1),
                        )
                    nc.scalar.copy(out=v_sb[:, tb, :], in_=pv[:, :])
                emit_b(0, *qk0)
                for li in range(1, HPC):
                    qk = emit_qk(li)
                    emit_b(li, *qk)

            # ---- out projection: transpose att, then attT.T @ Wp rows ----
            with tc.tile_pool(name="psC", bufs=2, space="PSUM") as psC:
                attT = [
                    wpool.tile([128, T], F32R, tag="w", name=f"attT{i}")
                    for i in range(2)
                ]
                for tb in range(NT):
                    for hc in range(2):
                        pt = psC.tile([128, 128], F32R, tag="t")
                        nc.tensor.transpose(
                            pt[:, :],
                            in_=v_sb[:, tb, hc * 128:(hc + 1) * 128],
                            identity=ident_r[:, :],
                        )
                        nc.vector.tensor_copy(attT[hc][:, tb * 128:(tb + 1) * 128], pt[:, :])
                for tb in range(NT):
                    po = psC.tile([128, C], F32, tag="o")
                    for nn_ in range(2):
                        for hc in range(2):
                            nc.tensor.matmul(
                                po[:, nn_ * 512:(nn_ + 1) * 512],
                                lhsT=attT[hc][:, tb * 128:(tb + 1) * 128],
                                rhs=wp_sb[:, hc, nn_ * 512:(nn_ + 1) * 512],
                                start=(hc == 0),
                                stop=(hc == 1),
                            )
                    osb = opool.tile([128, C], F32, tag="osb")
                    if tb % 2 == 0:
                        nc.scalar.copy(out=osb[:, :], in_=po[:, :])
                    else:
                        nc.vector.tensor_copy(osb[:, :], po[:, :])
                    nc.sync.dma_start(out=pout[tb * 128:(tb + 1) * 128, :], in_=osb[:, :])

    nc.compile()
    return nc


def _bf16_split3(r):
    """r (f32 vector) -> 3 rows exactly representable in bf16 summing ~r."""
    hi = r.astype(ml_dtypes.bfloat16).astype(np.float32)
    lo = (r - hi).astype(ml_dtypes.bfloat16).astype(np.float32)
    lolo = (r - hi - lo).astype(ml_dtypes.bfloat16).astype(np.float32)
    return np.stack([hi, lo, lolo])


def core_heads(c):
    return [4 * (c % 4) + i for i in range(HPC)]


def make_in_maps(x, Wq, Wk, Wv, Wp):
    j = np.arange(T, dtype=np.float64)
    p = np.arange(128, dtype=np.float64)
    mtri = np.where(
        np.arange(128)[None, :] <= np.arange(128)[:, None], 0.0, NEG
    ).astype(np.float32)
    ident = np.eye(128, dtype=np.float32)
    in_maps = []
    for c in range(NCORES):
        b = c // 4
        hs = core_heads(c)
        def swz(a, nch):
            # [nch*128, F] -> [128, nch*F] matching SBUF [p, chunk, free]
            f = a.shape[1]
            return np.ascontiguousarray(
                a.reshape(nch, 128, f).transpose(1, 0, 2).reshape(128, nch * f)
            )

        xt = swz(x[b].T, NCC)
        wqk_c = swz(
            np.concatenate(
                [np.concatenate([Wk[h].T, Wq[h].T], axis=1) for h in hs], axis=1
            ),
            NCC,
        )
        wv_c = swz(np.concatenate([Wv[h].T for h in hs], axis=1), NCC)
        wp_c = swz(
            np.concatenate([Wp[:, h * HD:(h + 1) * HD].T for h in hs], axis=0), 2
        )
        brows = np.zeros((3 * HPC, T), np.float32)
        bsi = np.zeros((128, HPC * NT), np.float32)
        for li, h in enumerate(hs):
            slope = float(2.0 ** (h * (-8.0 / H)))
            brows[3 * li:3 * li + 3] = _bf16_split3((slope * j).astype(np.float32))
            for k in range(NT):
                bsi[:, li * NT + k] = (-slope * (128.0 * k + p)).astype(np.float32)
        in_maps.append(
            {
                "xt": xt,
                "wqk": wqk_c,
                "wv": wv_c,
                "wp": wp_c,
                "brows": brows,
                "bsi": bsi,
                "mtri": mtri,
                "ident": ident,
                "identr": ident,
                "ones3": np.ones((3, T), np.float32),
            }
        )
    return in_maps


def assemble(results, bp):
    weights = np.zeros((B, H, T, T), np.float32)
    out = np.zeros((B, T, C), np.float32)
    for c in range(NCORES):
        b = c // 4
        for li, h in enumerate(core_heads(c)):
            weights[b, h] = results[c]["wout"][li]
        out[b] += results[c]["pout"]
    out += np.asarray(bp, np.float32)[None, None, :]
    return out, weights


_NC_CACHE = None


def _get_nc():
    global _NC_CACHE
    if _NC_CACHE is None:
        _NC_CACHE = build_nc()
    return _NC_CACHE


def kernel(x, Wq, Wk, Wv, Wp, bp):
    x = np.asarray(x, np.float32)
    Wq = np.asarray(Wq, np.float32)
    Wk = np.asarray(Wk, np.float32)
    Wv = np.asarray(Wv, np.float32)
    Wp = np.asarray(Wp, np.float32)
    nc = _get_nc()
    in_maps = make_in_maps(x, Wq, Wk, Wv, Wp)
    res = run_bass_kernel_spmd(nc, in_maps, core_ids=list(range(NCORES)))
    return assemble(res.results, bp)


# revision 19
# speedup vs baseline: 1.2281x; 1.0346x over previous
"""Trainium2 Bass kernel for nn_MultiHeadAttention_70712341561681.

Math (faithful to the reference):
  slopes[h] = 2^(-h/2)
  q,k,v   = per-head projections of x (no bias)
  logits[b,h,i,j] = k_i . q_j - slopes[h]*|i-j|   (rows i = key time)
  masked to j <= i, weights = softmax over j      -> OUTPUT 1 [B,H,T,T]
  att[b,h,t,:] = weights[b,h,t,t] * v[b,h,t,:]
  out = concat_heads(att) @ Wp.T + bp             -> OUTPUT 0 [B,T,C]

Sharding: 8 cores = 2 batches x 4 head-groups (4 heads each). Each core
computes its 4 [T,T] weight slices (lower triangle only; upper left zero
by the runtime's pre-zeroed output buffers) and a partial out projection
over its heads; host sums the 4 partials per batch (the "all-reduce").

Device kernel per core:
  - Q^T,K^T per head [64,T] via fp32r matmuls against x^T; 3 extra rows
    carry a bf16-split of slopes[h]*j so the logits matmul directly
    produces k.q + slope*j (softmax-invariant form; the -slope*i part is
    applied per-partition as the exp() bias on the scalar engine).
  - per 128-row tile: matmul chunks (only j <= i+127), -1e30 tril mask on
    the diagonal 128-block, exp with row bias + accumulated row sum,
    reciprocal, normalize, diagonal extract via identity multiply-reduce.
  - att = diag * v (in place), PE transpose, out-proj matmul.
"""

import os
import sys

import numpy as np

for _p in ("/opt/trn_rl_repo", "/root/.axon_site/_ro/trn_rl_repo"):
    if os.path.isdir(_p) and _p not in sys.path:
        sys.path.append(_p)

import ml_dtypes
import concourse.bacc as bacc
import concourse.mybir as mybir
from concourse.tile import TileContext
from concourse.bass_utils import run_bass_kernel_spmd

B, T, C, H = 2, 2048, 1024, 16
HD = C // H            # 64
NCORES = 8
HPC = 4                # heads per core
NT = T // 128          # 16 row-tiles
NCC = C // 128         # 8 contraction chunks
F32 = mybir.dt.float32
F32R = mybir.dt.float32r
EXP = mybir.ActivationFunctionType.Exp
MUL = mybir.AluOpType.mult
ADD = mybir.AluOpType.add
NEG = -1.0e30


def _r(ap):
    return ap.bitcast(F32R)


def build_nc():
    nc = bacc.Bacc(trn_type="TRN2")
    # pre-swizzled on host to the exact SBUF layout: [partition, chunk, free]
    xt = nc.dram_tensor("xt", [128, NCC * T], F32R, kind="ExternalInput")
    wqk = nc.dram_tensor("wqk", [128, NCC * HPC * 128], F32R, kind="ExternalInput")
    wv = nc.dram_tensor("wv", [128, NCC * HPC * HD], F32R, kind="ExternalInput")
    wp = nc.dram_tensor("wp", [128, 2 * C], F32R, kind="ExternalInput")
    brows = nc.dram_tensor("brows", [3 * HPC, T], F32R, kind="ExternalInput")
    bsi = nc.dram_tensor("bsi", [128, HPC * NT], F32, kind="ExternalInput")
    mtri = nc.dram_tensor("mtri", [128, 128], F32, kind="ExternalInput")
    ident = nc.dram_tensor("ident", [128, 128], F32, kind="ExternalInput")
    ones3 = nc.dram_tensor("ones3", [3, T], F32R, kind="ExternalInput")
    identr = nc.dram_tensor("identr", [128, 128], F32R, kind="ExternalInput")
    wout = nc.dram_tensor("wout", [HPC, T, T], F32, kind="ExternalOutput")
    pout = nc.dram_tensor("pout", [T, C], F32, kind="ExternalOutput")

    with TileContext(nc) as tc:
        with (
            tc.tile_pool(name="persist", bufs=1) as persist,
            tc.tile_pool(name="qpool", bufs=2) as qpool,
            tc.tile_pool(name="qscrp", bufs=1) as qscrp,
            tc.tile_pool(name="kpool", bufs=2) as kpool,
            tc.tile_pool(name="wpool", bufs=4) as wpool,
            tc.tile_pool(name="opool", bufs=2) as opool,
            tc.tile_pool(name="small", bufs=6) as small,
            tc.tile_pool(name="ttrp", bufs=2) as ttrp,
        ):
            # ---- resident loads (weights on the scalar HWDGE queue so the
            # sync queue is free for the big xt load) ----
            wqk_sb = persist.tile([128, NCC, HPC * 128], F32R)
            nc.scalar.dma_start(out=wqk_sb, in_=wqk[:, :].rearrange("p (cc m) -> p cc m", cc=NCC))
            wv_sb = persist.tile([128, NCC, HPC * HD], F32R)
            nc.scalar.dma_start(out=wv_sb, in_=wv[:, :].rearrange("p (cc m) -> p cc m", cc=NCC))
            wp_sb = persist.tile([128, 2, C], F32R)
            nc.scalar.dma_start(out=wp_sb, in_=wp[:, :].rearrange("p (hc co) -> p hc co", hc=2))
            bsi_sb = persist.tile([128, HPC * NT], F32)
            nc.scalar.dma_start(out=bsi_sb, in_=bsi[:, :])
            mtri_sb = persist.tile([128, 128], F32)
            nc.scalar.dma_start(out=mtri_sb, in_=mtri[:, :])
            ident_sb = persist.tile([128, 128], F32)
            nc.scalar.dma_start(out=ident_sb, in_=ident[:, :])
            ident_r = persist.tile([128, 128], F32R)
            nc.scalar.dma_start(out=ident_r, in_=identr[:, :])
            xt_sb = persist.tile([128, NCC, T], F32R)
            nc.sync.dma_start(out=xt_sb, in_=xt[:, :].rearrange("p (cc t) -> p cc t", cc=NCC))
            v_sb = persist.tile([128, NT, HPC * HD], F32R)
            diag_sb = persist.tile([128, HPC * NT], F32)

            with tc.tile_pool(name="psA", bufs=4, space="PSUM") as psA:

                def emit_qk(li):
                    qtext = qpool.tile([67, T], F32R, tag="qt", name=f"qt{li}")
                    ktext = kpool.tile([67, T], F32R, tag="kt", name=f"kt{li}")
                    qscr = qscrp.tile([128, T], F32R, tag="qscr", name=f"qs{li}")
                    nc.sync.dma_start(out=qtext[64:67, :], in_=brows[3 * li:3 * li + 3, :])
                    nc.sync.dma_start(out=ktext[64:67, :], in_=ones3[:, :])
                    # psum rows 0-63 = K^T, rows 64-127 = Q^T; two 1024 halves
                    for h2 in range(2):
                        ps = psA.tile([128, T // 2], F32, tag="u", name=f"ps{li}_{h2}")
                        for cc in range(NCC):
                            for j2 in range(2):
                                c0 = h2 * 1024 + j2 * 512
                                nc.tensor.matmul(
                                    ps[:, j2 * 512:(j2 + 1) * 512],
                                    lhsT=wqk_sb[:, cc, li * 128:(li + 1) * 128],
                                    rhs=xt_sb[:, cc, c0:c0 + 512],
                                    start=(cc == 0),
                                    stop=(cc == NCC - 1),
                                )
                        lo, hi = h2 * 1024, (h2 + 1) * 1024
                        if li % 2 == 0:
                            nc.scalar.copy(out=ktext[0:64, lo:hi], in_=ps[0:64, :])
                            nc.vector.tensor_copy(qscr[64:128, lo:hi], ps[64:128, :])
                        else:
                            nc.vector.tensor_copy(ktext[0:64, lo:hi], ps[0:64, :])
                            nc.scalar.copy(out=qscr[64:128, lo:hi], in_=ps[64:128, :])
                        # partition shift 64->0 via SBUF->SBUF DMA, per half
                        nc.sync.dma_start(
                            out=qtext[0:64, lo:hi], in_=qscr[64:128, lo:hi]
                        )
                    return qtext, ktext

                def emit_b(li, qtext, ktext):
                    for k in range(NT):
                        i0 = k * 128
                        ln = i0 + 128
                        wsb = wpool.tile([128, T], F32, tag="w", name=f"w{li}_{k}")
                        col = li * NT + k
                        nhalf = (ln + 1023) // 1024
                        ssum = small.tile([128, 2], F32, tag="ssum", name=f"ss{li}_{k}")
                        for hf in range(nhalf):
                            c0 = hf * 1024
                            c1 = min(ln, c0 + 1024)
                            pg = psA.tile([128, T // 2], F32, tag="u", name=f"pg{li}_{k}_{hf}")
                            for jc in range((c1 - c0 + 511) // 512):
                                n0 = c0 + jc * 512
                                n1 = min(c1, n0 + 512)
                                nc.tensor.matmul(
                                    pg[:, n0 - c0:n1 - c0],
                                    lhsT=ktext[0:67, i0:i0 + 128],
                                    rhs=qtext[0:67, n0:n1],
                                    start=True,
                                    stop=True,
                                )
                            if i0 >= c0 and i0 < c1:
                                # causal mask on the diagonal 128-block
                                nc.vector.tensor_add(
                                    pg[:, i0 - c0:i0 - c0 + 128],
                                    pg[:, i0 - c0:i0 - c0 + 128],
                                    mtri_sb[:, :],
                                )
                            nc.scalar.activation(
                                out=wsb[:, c0:c1],
                                in_=pg[:, 0:c1 - c0],
                                func=EXP,
                                bias=bsi_sb[:, col:col + 1],
                                scale=1.0,
                                accum_out=ssum[:, hf:hf + 1],
                            )
                        rec = small.tile([128, 1], F32, tag="rec", name=f"rc{li}_{k}")
                        if nhalf == 2:
                            stot = small.tile([128, 1], F32, tag="stot", name=f"st{li}_{k}")
                            nc.vector.tensor_add(stot[:, :], ssum[:, 0:1], ssum[:, 1:2])
                            nc.vector.reciprocal(rec[:, :], stot[:, :])
                        else:
                            nc.vector.reciprocal(rec[:, :], ssum[:, 0:1])
                        du = small.tile([128, 1], F32, tag="du", name=f"du{li}_{k}")
                        scr = ttrp.tile([128, 128], F32, tag="scr", name=f"sc{li}_{k}")
                        # diag numerator from the unnormalized diag block
                        nc.vector.tensor_mul(scr[:, :], wsb[:, i0:ln], ident_sb[:, :])
                        nc.vector.tensor_scalar_mul(wsb[:, 0:ln], wsb[:, 0:ln], rec[:, :])
                        nc.gpsimd.dma_start(out=wout[li, i0:i0 + 128, 0:ln], in_=wsb[:, 0:ln])
                        nc.vector.tensor_reduce(
                            out=du[:, :], in_=scr[:, :],
                            axis=mybir.AxisListType.X, op=ADD,
                        )
                        nc.vector.tensor_mul(diag_sb[:, col:col + 1], du[:, :], rec[:, :])

                    # att = diag * v for this head (in place on v_sb)
                    for tb in range(NT):
                        nc.vector.tensor_scalar_mul(
                            v_sb[:, tb, li * HD:(li + 1) * HD],
                            v_sb[:, tb, li * HD:(li + 1) * HD],
                            diag_sb[:, li * NT + tb:li * NT + tb + 1],
                        )

                # head 0's Q/K first so its softmax chain starts immediately;
                # the V projection then fills PE gaps during head 0's
                # vector/scalar-heavy phase.
                qk0 = emit_qk(0)
                for tb in range(NT):
                    pv = psA.tile([128, HPC * HD], F32, tag="u", name=f"pv{tb}")
                    for cc in range(NCC):
                        nc.tensor.matmul(
                            pv[:, :],
                            lhsT=xt_sb[:, cc, tb * 128:(tb + 1) * 128],
                            rhs=wv_sb[:, cc, :],
                            start=(cc == 0),
                            stop=(cc == NCC - 1),
                        )
                    nc.scalar.copy(out=v_sb[:, tb, :], in_=pv[:, :])
                emit_b(0, *qk0)
                for li in range(1, HPC):
                    qk = emit_qk(li)
                    emit_b(li, *qk)

            # ---- out projection: transpose att, then attT.T @ Wp rows ----
            with tc.tile_pool(name="psC", bufs=2, space="PSUM") as psC:
                attT = [
                    wpool.tile([128, T], F32R, tag="w", name=f"attT{i}")
                    for i in range(2)
                ]
                for tb in range(NT):
                    for hc in range(2):
                        pt = psC.tile([128, 128], F32R, tag="t")
                        nc.tensor.transpose(
                            pt[:, :],
                            in_=v_sb[:, tb, hc * 128:(hc + 1) * 128],
                            identity=ident_r[:, :],
                        )
                        nc.vector.tensor_copy(attT[hc][:, tb * 128:(tb + 1) * 128], pt[:, :])
                for tb in range(NT):
                    po = psC.tile([128, C], F32, tag="o")
                    for nn_ in range(2):
                        for hc in range(2):
                            nc.tensor.matmul(
                                po[:, nn_ * 512:(nn_ + 1) * 512],
                                lhsT=attT[hc][:, tb * 128:(tb + 1) * 128],
                                rhs=wp_sb[:, hc, nn_ * 512:(nn_ + 1) * 512],
                                start=(hc == 0),
                                stop=(hc == 1),
                            )
                    osb = opool.tile([128, C], F32, tag="osb")
                    if tb % 2 == 0:
                        nc.scalar.copy(out=osb[:, :], in_=po[:, :])
                    else:
                        nc.vector.tensor_copy(osb[:, :], po[:, :])
                    nc.sync.dma_start(out=pout[tb * 128:(tb + 1) * 128, :], in_=osb[:, :])

    nc.compile()
    return nc


def _bf16_split3(r):
    """r (f32 vector) -> 3 rows exactly representable in bf16 summing ~r."""
    hi = r.astype(ml_dtypes.bfloat16).astype(np.float32)
    lo = (r - hi).astype(ml_dtypes.bfloat16).astype(np.float32)
    lolo = (r - hi - lo).astype(ml_dtypes.bfloat16).astype(np.float32)
    return np.stack([hi, lo, lolo])


def core_heads(c):
    return [4 * (c % 4) + i for i in range(HPC)]


def make_in_maps(x, Wq, Wk, Wv, Wp):
    j = np.arange(T, dtype=np.float64)
    p = np.arange(128, dtype=np.float64)
    mtri = np.where(
        np.arange(128)[None, :] <= np.arange(128)[:, None], 0.0, NEG
    ).astype(np.float32)
    ident = np.eye(128, dtype=np.float32)
    in_maps = []
    for c in range(NCORES):
        b = c // 4
        hs = core_heads(c)
        def swz(a, nch):
            # [nch*128, F] -> [128, nch*F] matching SBUF [p, chunk, free]
            f = a.shape[1]
            return np.ascontiguousarray(
                a.reshape(nch, 128, f).transpose(1, 0, 2).reshape(128, nch * f)
            )

        xt = swz(x[b].T, NCC)
        wqk_c = swz(
            np.concatenate(
                [np.concatenate([Wk[h].T, Wq[h].T], axis=1) for h in hs], axis=1
            ),
            NCC,
        )
        wv_c = swz(np.concatenate([Wv[h].T for h in hs], axis=1), NCC)
        wp_c = swz(
            np.concatenate([Wp[:, h * HD:(h + 1) * HD].T for h in hs], axis=0), 2
        )
        brows = np.zeros((3 * HPC, T), np.float32)
        bsi = np.zeros((128, HPC * NT), np.float32)
        for li, h in enumerate(hs):
            slope = float(2.0 ** (h * (-8.0 / H)))
            brows[3 * li:3 * li + 3] = _bf16_split3((slope * j).astype(np.float32))
            for k in range(NT):
                bsi[:, li * NT + k] = (-slope * (128.0 * k + p)).astype(np.float32)
        in_maps.append(
            {
                "xt": xt,
                "wqk": wqk_c,
                "wv": wv_c,
                "wp": wp_c,
                "brows": brows,
                "bsi": bsi,
                "mtri": mtri,
                "ident": ident,
                "identr": ident,
                "ones3": np.ones((3, T), np.float32),
            }
        )
    return in_maps


def assemble(results, bp):
    weights = np.zeros((B, H, T, T), np.float32)
    out = np.zeros((B, T, C), np.float32)
    for c in range(NCORES):
        b = c // 4
        for li, h in enumerate(core_heads(c)):
            weights[b, h] = results[c]["wout"][li]
        out[b] += results[c]["pout"]
    out += np.asarray(bp, np.float32)[None, None, :]
    return out, weights


_NC_CACHE = None


def _get_nc():
    global _NC_CACHE
    if _NC_CACHE is None:
        _NC_CACHE = build_nc()
    return _NC_CACHE


def kernel(x, Wq, Wk, Wv, Wp, bp):
    x = np.asarray(x, np.float32)
    Wq = np.asarray(Wq, np.float32)
    Wk = np.asarray(Wk, np.float32)
    Wv = np.asarray(Wv, np.float32)
    Wp = np.asarray(Wp, np.float32)
    nc = _get_nc()
    in_maps = make_in_maps(x, Wq, Wk, Wv, Wp)
    res = run_bass_kernel_spmd(nc, in_maps, core_ids=list(range(NCORES)))
    return assemble(res.results, bp)


# revision 20
# speedup vs baseline: 1.2355x; 1.0061x over previous
"""Trainium2 Bass kernel for nn_MultiHeadAttention_70712341561681.

Math (faithful to the reference):
  slopes[h] = 2^(-h/2)
  q,k,v   = per-head projections of x (no bias)
  logits[b,h,i,j] = k_i . q_j - slopes[h]*|i-j|   (rows i = key time)
  masked to j <= i, weights = softmax over j      -> OUTPUT 1 [B,H,T,T]
  att[b,h,t,:] = weights[b,h,t,t] * v[b,h,t,:]
  out = concat_heads(att) @ Wp.T + bp             -> OUTPUT 0 [B,T,C]

Sharding: 8 cores = 2 batches x 4 head-groups (4 heads each). Each core
computes its 4 [T,T] weight slices (lower triangle only; upper left zero
by the runtime's pre-zeroed output buffers) and a partial out projection
over its heads; host sums the 4 partials per batch (the "all-reduce").

Device kernel per core:
  - Q^T,K^T per head [64,T] via fp32r matmuls against x^T; 3 extra rows
    carry a bf16-split of slopes[h]*j so the logits matmul directly
    produces k.q + slope*j (softmax-invariant form; the -slope*i part is
    applied per-partition as the exp() bias on the scalar engine).
  - per 128-row tile: matmul chunks (only j <= i+127), -1e30 tril mask on
    the diagonal 128-block, exp with row bias + accumulated row sum,
    reciprocal, normalize, diagonal extract via identity multiply-reduce.
  - att = diag * v (in place), PE transpose, out-proj matmul.
"""

import os
import sys

import numpy as np

for _p in ("/opt/trn_rl_repo", "/root/.axon_site/_ro/trn_rl_repo"):
    if os.path.isdir(_p) and _p not in sys.path:
        sys.path.append(_p)

import ml_dtypes
import concourse.bacc as bacc
import concourse.mybir as mybir
from concourse.tile import TileContext
from concourse.bass_utils import run_bass_kernel_spmd

B, T, C, H = 2, 2048, 1024, 16
HD = C // H            # 64
NCORES = 8
HPC = 4                # heads per core
NT = T // 128          # 16 row-tiles
NCC = C // 128         # 8 contraction chunks
F32 = mybir.dt.float32
F32R = mybir.dt.float32r
EXP = mybir.ActivationFunctionType.Exp
MUL = mybir.AluOpType.mult
ADD = mybir.AluOpType.add
NEG = -1.0e30


def _r(ap):
    return ap.bitcast(F32R)


def build_nc():
    nc = bacc.Bacc(trn_type="TRN2")
    # pre-swizzled on host to the exact SBUF layout: [partition, chunk, free]
    xt = nc.dram_tensor("xt", [128, NCC * T], F32R, kind="ExternalInput")
    wqk = nc.dram_tensor("wqk", [128, NCC * HPC * 128], F32R, kind="ExternalInput")
    wv = nc.dram_tensor("wv", [128, NCC * HPC * HD], F32R, kind="ExternalInput")
    wp = nc.dram_tensor("wp", [128, 2 * C], F32R, kind="ExternalInput")
    brows = nc.dram_tensor("brows", [3 * HPC, T], F32R, kind="ExternalInput")
    bsi = nc.dram_tensor("bsi", [128, HPC * NT], F32, kind="ExternalInput")
    mtri = nc.dram_tensor("mtri", [128, 128], F32, kind="ExternalInput")
    ident = nc.dram_tensor("ident", [128, 128], F32, kind="ExternalInput")
    ones3 = nc.dram_tensor("ones3", [3, T], F32R, kind="ExternalInput")
    identr = nc.dram_tensor("identr", [128, 128], F32R, kind="ExternalInput")
    wout = nc.dram_tensor("wout", [HPC, T, T], F32, kind="ExternalOutput")
    pout = nc.dram_tensor("pout", [T, C], F32, kind="ExternalOutput")

    with TileContext(nc) as tc:
        with (
            tc.tile_pool(name="persist", bufs=1) as persist,
            tc.tile_pool(name="qpool", bufs=2) as qpool,
            tc.tile_pool(name="qscrp", bufs=1) as qscrp,
            tc.tile_pool(name="kpool", bufs=2) as kpool,
            tc.tile_pool(name="wpool", bufs=4) as wpool,
            tc.tile_pool(name="opool", bufs=2) as opool,
            tc.tile_pool(name="small", bufs=12) as small,
            tc.tile_pool(name="ttrp", bufs=4) as ttrp,
        ):
            # ---- resident loads (weights on the scalar HWDGE queue so the
            # sync queue is free for the big xt load) ----
            wqk_sb = persist.tile([128, NCC, HPC * 128], F32R)
            nc.scalar.dma_start(out=wqk_sb, in_=wqk[:, :].rearrange("p (cc m) -> p cc m", cc=NCC))
            wv_sb = persist.tile([128, NCC, HPC * HD], F32R)
            nc.scalar.dma_start(out=wv_sb, in_=wv[:, :].rearrange("p (cc m) -> p cc m", cc=NCC))
            wp_sb = persist.tile([128, 2, C], F32R)
            nc.scalar.dma_start(out=wp_sb, in_=wp[:, :].rearrange("p (hc co) -> p hc co", hc=2))
            bsi_sb = persist.tile([128, HPC * NT], F32)
            nc.scalar.dma_start(out=bsi_sb, in_=bsi[:, :])
            mtri_sb = persist.tile([128, 128], F32)
            nc.scalar.dma_start(out=mtri_sb, in_=mtri[:, :])
            ident_sb = persist.tile([128, 128], F32)
            nc.scalar.dma_start(out=ident_sb, in_=ident[:, :])
            ident_r = persist.tile([128, 128], F32R)
            nc.scalar.dma_start(out=ident_r, in_=identr[:, :])
            xt_sb = persist.tile([128, NCC, T], F32R)
            nc.sync.dma_start(out=xt_sb, in_=xt[:, :].rearrange("p (cc t) -> p cc t", cc=NCC))
            v_sb = persist.tile([128, NT, HPC * HD], F32R)
            diag_sb = persist.tile([128, HPC * NT], F32)

            with tc.tile_pool(name="psA", bufs=4, space="PSUM") as psA:

                def emit_qk(li):
                    qtext = qpool.tile([67, T], F32R, tag="qt", name=f"qt{li}")
                    ktext = kpool.tile([67, T], F32R, tag="kt", name=f"kt{li}")
                    qscr = qscrp.tile([128, T], F32R, tag="qscr", name=f"qs{li}")
                    nc.sync.dma_start(out=qtext[64:67, :], in_=brows[3 * li:3 * li + 3, :])
                    nc.sync.dma_start(out=ktext[64:67, :], in_=ones3[:, :])
                    # psum rows 0-63 = K^T, rows 64-127 = Q^T; two 1024 halves
                    for h2 in range(2):
                        ps = psA.tile([128, T // 2], F32, tag="u", name=f"ps{li}_{h2}")
                        for cc in range(NCC):
                            for j2 in range(2):
                                c0 = h2 * 1024 + j2 * 512
                                nc.tensor.matmul(
                                    ps[:, j2 * 512:(j2 + 1) * 512],
                                    lhsT=wqk_sb[:, cc, li * 128:(li + 1) * 128],
                                    rhs=xt_sb[:, cc, c0:c0 + 512],
                                    start=(cc == 0),
                                    stop=(cc == NCC - 1),
                                )
                        lo, hi = h2 * 1024, (h2 + 1) * 1024
                        if li % 2 == 0:
                            nc.scalar.copy(out=ktext[0:64, lo:hi], in_=ps[0:64, :])
                            nc.vector.tensor_copy(qscr[64:128, lo:hi], ps[64:128, :])
                        else:
                            nc.vector.tensor_copy(ktext[0:64, lo:hi], ps[0:64, :])
                            nc.scalar.copy(out=qscr[64:128, lo:hi], in_=ps[64:128, :])
                        # partition shift 64->0 via SBUF->SBUF DMA, per half
                        nc.sync.dma_start(
                            out=qtext[0:64, lo:hi], in_=qscr[64:128, lo:hi]
                        )
                    return qtext, ktext

                def emit_b(li, qtext, ktext):
                    for k in range(NT):
                        i0 = k * 128
                        ln = i0 + 128
                        wsb = wpool.tile([128, T], F32, tag="w", name=f"w{li}_{k}")
                        col = li * NT + k
                        nhalf = (ln + 1023) // 1024
                        ssum = small.tile([128, 2], F32, tag="ssum", name=f"ss{li}_{k}")
                        for hf in range(nhalf):
                            c0 = hf * 1024
                            c1 = min(ln, c0 + 1024)
                            pg = psA.tile([128, T // 2], F32, tag="u", name=f"pg{li}_{k}_{hf}")
                            for jc in range((c1 - c0 + 511) // 512):
                                n0 = c0 + jc * 512
                                n1 = min(c1, n0 + 512)
                                nc.tensor.matmul(
                                    pg[:, n0 - c0:n1 - c0],
                                    lhsT=ktext[0:67, i0:i0 + 128],
                                    rhs=qtext[0:67, n0:n1],
                                    start=True,
                                    stop=True,
                                )
                            if i0 >= c0 and i0 < c1:
                                # causal mask on the diagonal 128-block
                                nc.vector.tensor_add(
                                    pg[:, i0 - c0:i0 - c0 + 128],
                                    pg[:, i0 - c0:i0 - c0 + 128],
                                    mtri_sb[:, :],
                                )
                            nc.scalar.activation(
                                out=wsb[:, c0:c1],
                                in_=pg[:, 0:c1 - c0],
                                func=EXP,
                                bias=bsi_sb[:, col:col + 1],
                                scale=1.0,
                                accum_out=ssum[:, hf:hf + 1],
                            )
                        rec = small.tile([128, 1], F32, tag="rec", name=f"rc{li}_{k}")
                        if nhalf == 2:
                            stot = small.tile([128, 1], F32, tag="stot", name=f"st{li}_{k}")
                            nc.vector.tensor_add(stot[:, :], ssum[:, 0:1], ssum[:, 1:2])
                            nc.vector.reciprocal(rec[:, :], stot[:, :])
                        else:
                            nc.vector.reciprocal(rec[:, :], ssum[:, 0:1])
                        du = small.tile([128, 1], F32, tag="du", name=f"du{li}_{k}")
                        scr = ttrp.tile([128, 128], F32, tag="scr", name=f"sc{li}_{k}")
                        # diag numerator from the unnormalized diag block
                        nc.vector.tensor_mul(scr[:, :], wsb[:, i0:ln], ident_sb[:, :])
                        nc.vector.tensor_scalar_mul(wsb[:, 0:ln], wsb[:, 0:ln], rec[:, :])
                        nc.gpsimd.dma_start(out=wout[li, i0:i0 + 128, 0:ln], in_=wsb[:, 0:ln])
                        nc.vector.tensor_reduce(
                            out=du[:, :], in_=scr[:, :],
                            axis=mybir.AxisListType.X, op=ADD,
                        )
                        nc.vector.tensor_mul(diag_sb[:, col:col + 1], du[:, :], rec[:, :])

                    # att = diag * v for this head (in place on v_sb)
                    for tb in range(NT):
                        nc.vector.tensor_scalar_mul(
                            v_sb[:, tb, li * HD:(li + 1) * HD],
                            v_sb[:, tb, li * HD:(li + 1) * HD],
                            diag_sb[:, li * NT + tb:li * NT + tb + 1],
                        )

                # head 0's Q/K first so its softmax chain starts immediately;
                # the V projection then fills PE gaps during head 0's
                # vector/scalar-heavy phase.
                qk0 = emit_qk(0)
                for tb in range(NT):
                    pv = psA.tile([128, HPC * HD], F32, tag="u", name=f"pv{tb}")
                    for cc in range(NCC):
                        nc.tensor.matmul(
                            pv[:, :],
                            lhsT=xt_sb[:, cc, tb * 128:(tb + 1) * 128],
                            rhs=wv_sb[:, cc, :],
                            start=(cc == 0),
                            stop=(cc == NCC - 1),
                        )
                    nc.scalar.copy(out=v_sb[:, tb, :], in_=pv[:, :])
                emit_b(0, *qk0)
                for li in range(1, HPC):
                    qk = emit_qk(li)
                    emit_b(li, *qk)

            # ---- out projection: transpose att, then attT.T @ Wp rows ----
            with tc.tile_pool(name="psC", bufs=2, space="PSUM") as psC:
                attT = [
                    wpool.tile([128, T], F32R, tag="w", name=f"attT{i}")
                    for i in range(2)
                ]
                for tb in range(NT):
                    for hc in range(2):
                        pt = psC.tile([128, 128], F32R, tag="t")
                        nc.tensor.transpose(
                            pt[:, :],
                            in_=v_sb[:, tb, hc * 128:(hc + 1) * 128],
                            identity=ident_r[:, :],
                        )
                        nc.vector.tensor_copy(attT[hc][:, tb * 128:(tb + 1) * 128], pt[:, :])
                for tb in range(NT):
                    po = psC.tile([128, C], F32, tag="o")
                    for nn_ in range(2):
                        for hc in range(2):
                            nc.tensor.matmul(
                                po[:, nn_ * 512:(nn_ + 1) * 512],
                                lhsT=attT[hc][:, tb * 128:(tb + 1) * 128],
                                rhs=wp_sb[:, hc, nn_ * 512:(nn_ + 1) * 512],
                                start=(hc == 0),
                                stop=(hc == 1),
                            )
                    osb = opool.tile([128, C], F32, tag="osb")
                    if tb % 2 == 0:
                        nc.scalar.copy(out=osb[:, :], in_=po[:, :])
                    else:
                        nc.vector.tensor_copy(osb[:, :], po[:, :])
                    nc.sync.dma_start(out=pout[tb * 128:(tb + 1) * 128, :], in_=osb[:, :])

    nc.compile()
    return nc


def _bf16_split3(r):
    """r (f32 vector) -> 3 rows exactly representable in bf16 summing ~r."""
    hi = r.astype(ml_dtypes.bfloat16).astype(np.float32)
    lo = (r - hi).astype(ml_dtypes.bfloat16).astype(np.float32)
    lolo = (r - hi - lo).astype(ml_dtypes.bfloat16).astype(np.float32)
    return np.stack([hi, lo, lolo])


def core_heads(c):
    return [4 * (c % 4) + i for i in range(HPC)]


def make_in_maps(x, Wq, Wk, Wv, Wp):
    j = np.arange(T, dtype=np.float64)
    p = np.arange(128, dtype=np.float64)
    mtri = np.where(
        np.arange(128)[None, :] <= np.arange(128)[:, None], 0.0, NEG
    ).astype(np.float32)
    ident = np.eye(128, dtype=np.float32)
    in_maps = []
    for c in range(NCORES):
        b = c // 4
        hs = core_heads(c)
        def swz(a, nch):
            # [nch*128, F] -> [128, nch*F] matching SBUF [p, chunk, free]
            f = a.shape[1]
            return np.ascontiguousarray(
                a.reshape(nch, 128, f).transpose(1, 0, 2).reshape(128, nch * f)
            )

        xt = swz(x[b].T, NCC)
        wqk_c = swz(
            np.concatenate(
                [np.concatenate([Wk[h].T, Wq[h].T], axis=1) for h in hs], axis=1
            ),
            NCC,
        )
        wv_c = swz(np.concatenate([Wv[h].T for h in hs], axis=1), NCC)
        wp_c = swz(
            np.concatenate([Wp[:, h * HD:(h + 1) * HD].T for h in hs], axis=0), 2
        )
        brows = np.zeros((3 * HPC, T), np.float32)
        bsi = np.zeros((128, HPC * NT), np.float32)
        for li, h in enumerate(hs):
            slope = float(2.0 ** (h * (-8.0 / H)))
            brows[3 * li:3 * li + 3] = _bf16_split3((slope * j).astype(np.float32))
            for k in range(NT):
                bsi[:, li * NT + k] = (-slope * (128.0 * k + p)).astype(np.float32)
        in_maps.append(
            {
                "xt": xt,
                "wqk": wqk_c,
                "wv": wv_c,
                "wp": wp_c,
                "brows": brows,
                "bsi": bsi,
                "mtri": mtri,
                "ident": ident,
                "identr": ident,
                "ones3": np.ones((3, T), np.float32),
            }
        )
    return in_maps


def assemble(results, bp):
    weights = np.zeros((B, H, T, T), np.float32)
    out = np.zeros((B, T, C), np.float32)
    for c in range(NCORES):
        b = c // 4
        for li, h in enumerate(core_heads(c)):
            weights[b, h] = results[c]["wout"][li]
        out[b] += results[c]["pout"]
    out += np.asarray(bp, np.float32)[None, None, :]
    return out, weights


_NC_CACHE = None


def _get_nc():
    global _NC_CACHE
    if _NC_CACHE is None:
        _NC_CACHE = build_nc()
    return _NC_CACHE


def kernel(x, Wq, Wk, Wv, Wp, bp):
    x = np.asarray(x, np.float32)
    Wq = np.asarray(Wq, np.float32)
    Wk = np.asarray(Wk, np.float32)
    Wv = np.asarray(Wv, np.float32)
    Wp = np.asarray(Wp, np.float32)
    nc = _get_nc()
    in_maps = make_in_maps(x, Wq, Wk, Wv, Wp)
    res = run_bass_kernel_spmd(nc, in_maps, core_ids=list(range(NCORES)))
    return assemble(res.results, bp)
